# revision 8
# baseline (speedup 1.0000x reference)
import sys
sys.path.insert(0, '/opt/trn_rl_repo')
import numpy as np

P = 128
NCORES = 8
F = 128


def _build_bass(n_pad, nblk, K, weights, biases, Wp, bp):
    """Build the SPMD Bass program. Identical trace for all cores; per-core
    data arrives via ExternalInputs (x slice + edge metadata)."""
    from concourse import bass, bacc, mybir
    import concourse.tile as tile

    slice_rows = nblk * P
    ncols = nblk * K

    nc = bacc.Bacc(num_devices=NCORES, num_swdge_queues=4)

    x_in = nc.declare_dram_parameter("x_in", [slice_rows, F], mybir.dt.float32, isOutput=False)
    midx_in = nc.declare_dram_parameter("midx", [P, ncols], mybir.dt.int32, isOutput=False)
    mdlc_in = nc.declare_dram_parameter("mdlc", [P, ncols], mybir.dt.float32, isOutput=False)
    mnrm_in = nc.declare_dram_parameter("mnrm", [P, ncols], mybir.dt.float32, isOutput=False)
    y_out = nc.declare_dram_parameter("y_out", [2, slice_rows], mybir.dt.float32, isOutput=True)

    # internal DRAM
    ag_in_x = nc.dram_tensor("ag_in_x", [slice_rows, F], mybir.dt.float32)
    h_slice = [nc.dram_tensor(f"h_slice{i}", [slice_rows, F], mybir.dt.float32) for i in range(3)]
    v_full = [nc.dram_tensor(f"v_full{i}", [n_pad, F], mybir.dt.float32, addr_space="Shared")
              for i in range(4)]

    # inline constants (same on every core)
    iota_np = np.broadcast_to(np.arange(P, dtype=np.float32), (P, P)).copy()
    iota_d = nc.inline_tensor(iota_np, name="iota_c")
    W_d = [nc.inline_tensor(np.ascontiguousarray(w, dtype=np.float32), name=f"W{i}")
           for i, w in enumerate(weights)]
    B_d = [nc.inline_tensor(np.broadcast_to(b.astype(np.float32), (P, F)).copy(), name=f"B{i}")
           for i, b in enumerate(biases)]
    Wp_d = [nc.inline_tensor(np.ascontiguousarray(Wp[i*F:(i+1)*F, :], dtype=np.float32), name=f"Wp{i}")
            for i in range(3)]
    bp_d = nc.inline_tensor(np.ascontiguousarray(bp.reshape(2, 1), dtype=np.float32), name="bp_c")

    AF = mybir.ActivationFunctionType
    ALU = mybir.AluOpType
    rg = [list(range(NCORES))]

    with tile.TileContext(nc) as tc:
        with (
            tc.tile_pool(name="const", bufs=1) as cpool,
            tc.tile_pool(name="work", bufs=8) as pool,
            tc.tile_pool(name="epi", bufs=1) as epool,
            tc.tile_pool(name="psum", bufs=3, space="PSUM") as psum,
            tc.tile_pool(name="psum2", bufs=2, space="PSUM") as psum2,
        ):
            # ---- prologue: stage x slice, allgather to x_full ----
            nc.sync.dma_start(out=ag_in_x[:, :], in_=x_in[:, :])
            nc.gpsimd.collective_compute(
                "AllGather", ALU.bypass, replica_groups=rg,
                ins=[ag_in_x[:].opt()], outs=[v_full[0][:].opt()],
            )

            iota_t = cpool.tile([P, P], mybir.dt.float32)
            nc.sync.dma_start(out=iota_t[:], in_=iota_d[:, :])
            W_t = []
            B_t = []
            Wp_t = []
            for i in range(3):
                wt = cpool.tile([P, F], mybir.dt.float32, tag=f"w{i}")
                nc.sync.dma_start(out=wt[:], in_=W_d[i][:, :])
                W_t.append(wt)
                bt = cpool.tile([P, F], mybir.dt.float32, tag=f"b{i}")
                nc.sync.dma_start(out=bt[:], in_=B_d[i][:, :])
                B_t.append(bt)
                wpt = cpool.tile([P, 2], mybir.dt.float32, tag=f"wp{i}")
                nc.sync.dma_start(out=wpt[:], in_=Wp_d[i][:, :])
                Wp_t.append(wpt)
            bp_t = cpool.tile([2, 1], mybir.dt.float32)
            nc.sync.dma_start(out=bp_t[:], in_=bp_d[:, :])

            midx_t = cpool.tile([P, ncols], mybir.dt.int32)
            mdlc_t = cpool.tile([P, ncols], mybir.dt.float32)
            mnrm_t = cpool.tile([P, ncols], mybir.dt.float32)
            nc.sync.dma_start(out=midx_t[:], in_=midx_in[:, :])
            nc.sync.dma_start(out=mdlc_t[:], in_=mdlc_in[:, :])
            nc.sync.dma_start(out=mnrm_t[:], in_=mnrm_in[:, :])

            yT_acc = cpool.tile([2, slice_rows], mybir.dt.float32)
            nc.vector.memset(yT_acc[:], 0.0)
            hsl = cpool.tile([P, slice_rows], mybir.dt.float32)

            # ---- 4 propagations ----
            for i in range(4):
                src = v_full[i]
                for b in range(nblk):
                    gt = psum.tile([P, P], mybir.dt.float32, tag="gt", space="PSUM")
                    for k in range(K):
                        col = b * K + k
                        msg = pool.tile([P, F], mybir.dt.float32, tag="msg")
                        gins = nc.gpsimd.indirect_dma_start(
                            out=msg[:], out_offset=None,
                            in_=src[:],
                            in_offset=bass.IndirectOffsetOnAxis(
                                ap=midx_t[:, col:col+1], axis=0),
                        )
                        qn = (i * nblk * K + col) % 4
                        gins.ins.queue = f"qPoolDynamic{qn or ''}"''
                        S = pool.tile([P, P], mybir.dt.float32, tag="S")
                        nc.vector.tensor_scalar(
                            out=S[:], in0=iota_t[:],
                            scalar1=mdlc_t[:, col:col+1], op0=ALU.is_equal,
                            scalar2=mnrm_t[:, col:col+1], op1=ALU.mult,
                        )
                        nc.tensor.matmul(out=gt[:], lhsT=msg[:], rhs=S[:],
                                         start=(k == 0), stop=(k == K - 1))
                    # gt = gT block [f, d]
                    gts = pool.tile([P, P], mybir.dt.float32, tag="gts")
                    nc.scalar.copy(out=gts[:], in_=gt[:])
                    if i < 3:
                        hp = psum2.tile([P, P], mybir.dt.float32, tag="hp", space="PSUM")
                        nc.tensor.matmul(out=hp[:], lhsT=gts[:], rhs=W_t[i][:],
                                         start=True, stop=True)
                        hb = hsl[:, b*P:(b+1)*P]
                        nc.vector.tensor_tensor(out=hb, in0=hp[:], in1=B_t[i][:], op=ALU.add)
                        nc.vector.tensor_scalar_max(out=hb, in0=hb, scalar1=0.0)
                    if i >= 1:
                        yp = psum2.tile([2, P], mybir.dt.float32, tag="yp", space="PSUM")
                        nc.tensor.matmul(out=yp[:], lhsT=Wp_t[i-1][:], rhs=gts[:],
                                         start=True, stop=True)
                        nc.vector.tensor_tensor(
                            out=yT_acc[:, b*P:(b+1)*P],
                            in0=yT_acc[:, b*P:(b+1)*P], in1=yp[:], op=ALU.add)
                if i < 3:
                    # single funnel DMA: hsl [P, nblk*P] -> h_slice rows (node-major)
                    nc.sync.dma_start(
                        out=h_slice[i].rearrange("(b d) o -> d b o", d=P),
                        in_=hsl[:].rearrange("d (b o) -> d b o", o=P))
                    nc.gpsimd.collective_compute(
                        "AllGather", ALU.bypass, replica_groups=rg,
                        ins=[h_slice[i][:].opt()], outs=[v_full[i+1][:].opt()],
                    )

            # ---- epilogue: y = yT_acc + bp; softmax(2) = sigmoid(y0-y1) ----
            nc.vector.tensor_scalar_add(out=yT_acc[:], in0=yT_acc[:],
                                        scalar1=bp_t[:, 0:1])
            nchunk_e = 4
            cw = slice_rows // nchunk_e
            for ce in range(nchunk_e):
                cs = ce * cw
                t1 = epool.tile([1, cw], mybir.dt.float32, tag="t1")
                nc.sync.dma_start(out=t1[:], in_=yT_acc[1:2, cs:cs+cw])
                dif = epool.tile([1, cw], mybir.dt.float32, tag="dif")
                nc.vector.tensor_tensor(out=dif[:], in0=yT_acc[0:1, cs:cs+cw],
                                        in1=t1[:], op=ALU.subtract)
                sig = epool.tile([1, cw], mybir.dt.float32, tag="sig")
                nc.scalar.activation(out=sig[:], in_=dif[:], func=AF.Sigmoid)
                om = epool.tile([1, cw], mybir.dt.float32, tag="om")
                nc.vector.tensor_scalar(out=om[:], in0=sig[:],
                                        scalar1=-1.0, op0=ALU.mult,
                                        scalar2=1.0, op1=ALU.add)
                nc.sync.dma_start(out=y_out[0:1, cs:cs+cw], in_=sig[:])
                nc.sync.dma_start(out=y_out[1:2, cs:cs+cw], in_=om[:])

    nc.compile()
    return nc


def _prep(x, edge_index, n_pad, nblk_per_core):
    """Host-side graph prep: gcn norm, dest-sorted per-core chunk metadata."""
    N = x.shape[0]
    E = edge_index.shape[1]
    row = np.concatenate([edge_index[0].astype(np.int64), np.arange(N, dtype=np.int64)])
    col = np.concatenate([edge_index[1].astype(np.int64), np.arange(N, dtype=np.int64)])
    deg = np.bincount(col, minlength=N).astype(np.float32)
    dis = np.where(deg > 0, 1.0 / np.sqrt(deg), 0.0).astype(np.float32)
    norm = dis[row] * dis[col]

    order = np.argsort(col, kind='stable')
    row_s = row[order].astype(np.int32)
    col_s = col[order].astype(np.int32)
    nrm_s = norm[order]

    slice_rows = nblk_per_core * P
    blk_of_edge = col_s // P              # global block id per edge
    nblk_tot = NCORES * nblk_per_core
    cnt = np.bincount(blk_of_edge, minlength=nblk_tot)
    K = int(np.ceil(cnt.max() / P))
    ncols = nblk_per_core * K

    blk_starts = np.zeros(nblk_tot + 1, np.int64)
    np.cumsum(cnt, out=blk_starts[1:])

    metas = []
    for c in range(NCORES):
        midx = np.zeros((P, ncols), np.int32)
        mdlc = np.zeros((P, ncols), np.float32)
        mnrm = np.zeros((P, ncols), np.float32)
        for j in range(nblk_per_core):
            g = c * nblk_per_core + j
            s, e = blk_starts[g], blk_starts[g + 1]
            n = e - s
            npad = K * P
            src_p = np.zeros(npad, np.int32)
            dlc_p = np.zeros(npad, np.float32)
            nrm_p = np.zeros(npad, np.float32)
            src_p[:n] = row_s[s:e]
            dlc_p[:n] = (col_s[s:e] - g * P).astype(np.float32)
            nrm_p[:n] = nrm_s[s:e]
            # edge t of block -> chunk t//P, partition t%P ; column j*K + chunk
            midx[:, j*K:(j+1)*K] = src_p.reshape(K, P).T
            mdlc[:, j*K:(j+1)*K] = dlc_p.reshape(K, P).T
            mnrm[:, j*K:(j+1)*K] = nrm_p.reshape(K, P).T
        metas.append((midx, mdlc, mnrm))

    x_pad = np.zeros((n_pad, x.shape[1]), np.float32)
    x_pad[:N] = x
    x_slices = [x_pad[c*slice_rows:(c+1)*slice_rows] for c in range(NCORES)]
    return metas, x_slices, K


LAST_RESULTS = None
LAST_NC = None
LAST_IN_MAPS = None


def kernel(x, edge_index, W0, b0, W1, b1, W2, b2, Wp, bp):
    global LAST_RESULTS, LAST_NC, LAST_IN_MAPS
    import os
    from concourse.bass_utils import run_bass_kernel_spmd

    x = np.asarray(x, dtype=np.float32)
    edge_index = np.asarray(edge_index)
    N = x.shape[0]
    nblk_per_core = int(np.ceil(N / (NCORES * P)))
    n_pad = NCORES * nblk_per_core * P
    slice_rows = nblk_per_core * P

    metas, x_slices, K = _prep(x, edge_index, n_pad, nblk_per_core)

    nc = _build_bass(
        n_pad, nblk_per_core, K,
        [np.asarray(W0), np.asarray(W1), np.asarray(W2)],
        [np.asarray(b0), np.asarray(b1), np.asarray(b2)],
        np.asarray(Wp), np.asarray(bp),
    )

    in_maps = []
    for c in range(NCORES):
        midx, mdlc, mnrm = metas[c]
        in_maps.append({
            "x_in": np.ascontiguousarray(x_slices[c]),
            "midx": midx, "mdlc": mdlc, "mnrm": mnrm,
        })

    trace = bool(os.environ.get("KERNEL_TRACE"))
    res = run_bass_kernel_spmd(nc, in_maps, list(range(NCORES)), trace=trace)
    LAST_RESULTS = res
    LAST_NC = nc
    LAST_IN_MAPS = in_maps

    out = np.zeros((n_pad, 2), np.float32)
    for c in range(NCORES):
        yT = res.results[c]["y_out"]          # [2, slice_rows]
        out[c*slice_rows:(c+1)*slice_rows] = yT.T
    return out[:N]
